# revision 5
# baseline (speedup 1.0000x reference)
"""Trainium2 Bass kernel for per-pixel (untied) local depthwise conv.

Problem: out[n,h,w,c] = sum_{dh,dw} in[n, h+dh-2, w+dw-2, c] * wt[n, h, w, dh*5+dw]
Shapes: in (8,512,512,3) f32, wt (8,512,512,25) f32, 'same' zero padding.

Strategy (8 NeuronCores, data-parallel over N, one image per core):
  - Host pre-lays data into SBUF-ready fp16 buffers (layout + dtype work is
    host-side; device only does bulk contiguous DMA).
  - Layout: 128 partitions x 4 output rows each.  Each partition holds its
    4 rows plus a 2-row halo (8 padded rows) of the input, in channel-plane
    form, duplicated at two column parities so every tap's DVE read is
    4-byte aligned (keeps fp16 tensor_tensor in 2x perf mode).
  - Per W-tile (4 tiles of 128 cols): for each of the 25 taps, DVE computes
    the elementwise product (shifted input x per-pixel weight, weight
    broadcast over the 3 channels via a stride-0 AP dim) in fp16.
  - TensorE accumulates the 25 product tiles into PSUM (f32) with
    identity-stationary matmuls (start/stop accumulation flags) - this is
    the only engine that can accumulate, freeing DVE from the add passes.
  - ScalarE evacuates PSUM -> SBUF fp16, DMA out; host converts to f32.
"""

import sys

sys.path.insert(0, "/opt/trn_rl_repo")

import numpy as np

import concourse.bass as bass
import concourse.mybir as mybir
from concourse.tile import TileContext
from concourse.bass_utils import run_bass_kernel_spmd

N, H, W, C, K = 8, 512, 512, 3, 5
KK = K * K
N_CORES = 8
RPP = H // 128          # output rows per partition (4)
HALO_ROWS = RPP + K - 1  # input rows held per partition (8)
JW = 520                 # padded row width per copy (covers cols -2..517)
N_WT = 4                 # W tiles
WT_W = W // N_WT         # 128 cols per W tile
PF = C * RPP * WT_W      # product free size per partition (1536)
BANK = 512               # psum bank free size (f32)
NB = PF // BANK          # psum banks per W tile (3)
X_FREE = 2 * C * HALO_ROWS * JW   # 24960 fp16 elems per partition
WT_FREE = KK * RPP * WT_W         # 12800 fp16 elems per partition per W tile


def _split_multi_waits(nc):
    """The walrus build in this container encodes at most ONE sync-wait per
    instruction; Tile's sem-assignment attaches one wait per dependency.
    Hoist the extra waits onto single-wait NOPs inserted just before the
    instruction on the same engine (equivalent sequencer semantics)."""
    n_split = 0
    for f in nc.m.functions:
        for bb in f.blocks:
            new_insts = []
            changed = False
            for inst in bb.instructions:
                si = inst.sync_info
                waits = list(si.on_wait) if (si is not None and si.on_wait) else []
                if len(waits) > 1:
                    changed = True
                    for w in waits[:-1]:
                        nop = mybir.InstNoOp(
                            name=nc.get_next_instruction_name(),
                            engine=inst.engine,
                            sync_info=mybir.SyncInfo(on_wait=[w], on_update=[]),
                            bass_nofuse=True,
                        )
                        new_insts.append(nop)
                        n_split += 1
                    inst.sync_info = mybir.SyncInfo(
                        on_wait=[waits[-1]],
                        on_update=list(si.on_update) if si.on_update else [],
                    )
                new_insts.append(inst)
            if changed:
                bb.instructions = new_insts
    return n_split


_NC_CACHE = None


def _build_program():
    global _NC_CACHE
    if _NC_CACHE is not None:
        return _NC_CACHE

    fp16 = mybir.dt.float16
    f32 = mybir.dt.float32

    nc = bass.Bass("TRN2", target_bir_lowering=False, debug=False,
                   num_devices=N_CORES)
    xbuf = nc.dram_tensor("xbuf", [128, X_FREE], fp16, kind="ExternalInput").ap()
    wtbuf = nc.dram_tensor("wtbuf", [N_WT, 128, WT_FREE], fp16,
                           kind="ExternalInput").ap()
    ident = nc.dram_tensor("ident", [128, 128], fp16, kind="ExternalInput").ap()
    out = nc.dram_tensor("out", [N_WT, 128, PF], fp16, kind="ExternalOutput").ap()

    with TileContext(nc) as tc:
        with (
            tc.tile_pool(name="xpool", bufs=1) as xpool,
            tc.tile_pool(name="wtpool", bufs=2) as wtpool,
            tc.tile_pool(name="prodpool", bufs=6) as prodpool,
            tc.tile_pool(name="outpool", bufs=2) as outpool,
            tc.tile_pool(name="psumpool", bufs=2 * NB, space="PSUM") as psumpool,
        ):
            id_t = xpool.tile([128, 128], fp16)
            nc.sync.dma_start(out=id_t[:], in_=ident[:])
            x_t = xpool.tile([128, X_FREE], fp16)
            nc.sync.dma_start(out=x_t[:], in_=xbuf[:])
            # view: [p, parity, c, row, j]
            xv = x_t[:].rearrange("p (s c r j) -> p s c r j", s=2, c=C, r=HALO_ROWS)

            for t in range(N_WT):
                wt_t = wtpool.tile([128, WT_FREE], fp16)
                nc.sync.dma_start(out=wt_t[:], in_=wtbuf[t])

                ps = [
                    psumpool.tile([128, BANK], f32, tag="ps", name=f"ps_{t}_{b}")
                    for b in range(NB)
                ]

                for k in range(KK):
                    dh, dw = k // K, k % K
                    sel = dw % 2          # parity copy: even dw -> 0, odd -> 1
                    j0 = t * WT_W + dw + sel
                    prod = prodpool.tile([128, PF], fp16)
                    pv = prod[:].rearrange("p (c r w) -> p c r w", c=C, r=RPP)
                    xs = xv[:, sel, :, dh:dh + RPP, j0:j0 + WT_W]
                    wk = (
                        wt_t[:, k * RPP * WT_W:(k + 1) * RPP * WT_W]
                        .rearrange("p (r w) -> p r w", r=RPP)[:, None, :, :]
                        .broadcast_to([128, C, RPP, WT_W])
                    )
                    nc.vector.tensor_mul(out=pv, in0=xs, in1=wk)
                    for b in range(NB):
                        nc.tensor.matmul(
                            ps[b][:],
                            id_t[:],
                            prod[:, b * BANK:(b + 1) * BANK],
                            start=(k == 0),
                            stop=(k == KK - 1),
                        )

                o_t = outpool.tile([128, PF], fp16)
                for b in range(NB):
                    nc.scalar.copy(
                        out=o_t[:, b * BANK:(b + 1) * BANK], in_=ps[b][:]
                    )
                nc.sync.dma_start(out=out[t], in_=o_t[:])

    _split_multi_waits(nc)
    _NC_CACHE = nc
    return nc


def _pack_inputs(input_data: np.ndarray, weights: np.ndarray):
    """Host-side layout + fp16 conversion into per-core SBUF-ready buffers."""
    xh = input_data.astype(np.float16)     # (N, H, W, C)
    wh = weights.astype(np.float16)        # (N, H, W, KK)

    in_maps = []
    identity = np.eye(128, dtype=np.float16)
    for n in range(N_CORES):
        # canvas[c, rr, j]: rr = input row + 2 (0..515), col -3.. via j offset
        canvas = np.zeros((C, H + 4, JW + 1), dtype=np.float16)
        canvas[:, 2:2 + H, 3:3 + W] = xh[n].transpose(2, 0, 1)
        rows_a = canvas[:, :, 1:1 + JW]    # A[j] = col j-2
        rows_b = canvas[:, :, 0:JW]        # B[j] = col j-3
        # partition p, halo row rr -> canvas row 4p+rr
        swa = np.lib.stride_tricks.sliding_window_view(rows_a, HALO_ROWS, axis=1)
        swb = np.lib.stride_tricks.sliding_window_view(rows_b, HALO_ROWS, axis=1)
        # (C, 509, JW, 8) -> take every 4th start -> (C, 128, JW, 8)
        swa = swa[:, ::RPP][:, :128]
        swb = swb[:, ::RPP][:, :128]
        # -> (128, parity, C, rr, j)
        X = np.empty((128, 2, C, HALO_ROWS, JW), dtype=np.float16)
        X[:, 0] = swa.transpose(1, 0, 3, 2)
        X[:, 1] = swb.transpose(1, 0, 3, 2)
        X = np.ascontiguousarray(X.reshape(128, X_FREE))

        # WT[t, p, k, r, w] = wh[n, 4p+r, 128t+w, k]
        wv = wh[n].reshape(128, RPP, N_WT, WT_W, KK)
        WT = np.ascontiguousarray(
            wv.transpose(2, 0, 4, 1, 3).reshape(N_WT, 128, WT_FREE)
        )
        in_maps.append({"xbuf": X, "wtbuf": WT, "ident": identity})
    return in_maps


def _unpack_outputs(results) -> np.ndarray:
    out = np.empty((N, H, W, C), dtype=np.float32)
    for n in range(N_CORES):
        o = results[n]["out"].astype(np.float32)  # (N_WT, 128, PF)
        o = o.reshape(N_WT, 128, C, RPP, WT_W)
        # -> h = 4p+r, w = 128t+w, c
        out[n] = o.transpose(1, 3, 0, 4, 2).reshape(H, W, C)
    return out


def kernel(input_data: np.ndarray, weights: np.ndarray) -> np.ndarray:
    input_data = np.asarray(input_data, dtype=np.float32)
    weights = np.asarray(weights, dtype=np.float32)
    nc = _build_program()
    in_maps = _pack_inputs(input_data, weights)
    res = run_bass_kernel_spmd(nc, in_maps, list(range(N_CORES)))
    return _unpack_outputs(res.results)


if __name__ == "__main__":
    rng = np.random.default_rng(0)
    x = rng.standard_normal((N, H, W, C), dtype=np.float32)
    w = rng.standard_normal((N, H, W, KK), dtype=np.float32) * 0.1
    out = kernel(input_data=x, weights=w)
    print("out", out.shape, out.dtype, float(np.abs(out).mean()))


# revision 8
# speedup vs baseline: 1.0318x; 1.0318x over previous
"""Trainium2 Bass kernel for per-pixel (untied) local depthwise conv.

Problem: out[n,h,w,c] = sum_{dh,dw} in[n, h+dh-2, w+dw-2, c] * wt[n, h, w, dh*5+dw]
Shapes: in (8,512,512,3) f32, wt (8,512,512,25) f32, 'same' zero padding.

Strategy (8 NeuronCores, data-parallel over N, one image per core):
  - Host pre-lays data into SBUF-ready fp16 buffers (layout + dtype work is
    host-side; device only does bulk contiguous DMA).
  - Layout: 128 partitions x 4 output rows each.  Each partition holds its
    4 rows plus a 2-row halo (8 padded rows) of the input, in channel-plane
    form, duplicated at two column parities so every tap's DVE read is
    4-byte aligned (keeps fp16 tensor_tensor in 2x perf mode).
  - Per W-tile (4 tiles of 128 cols): for each of the 25 taps, DVE computes
    the elementwise product (shifted input x per-pixel weight, weight
    broadcast over the 3 channels via a stride-0 AP dim) in fp16.
  - TensorE accumulates the 25 product tiles into PSUM (f32) with
    identity-stationary matmuls (start/stop accumulation flags) - this is
    the only engine that can accumulate, freeing DVE from the add passes.
  - ScalarE evacuates PSUM -> SBUF fp16, DMA out; host converts to f32.
"""

import sys

sys.path.insert(0, "/opt/trn_rl_repo")

import numpy as np

import concourse.bass as bass
import concourse.mybir as mybir
from concourse.tile import TileContext
from concourse.bass_utils import run_bass_kernel_spmd

N, H, W, C, K = 8, 512, 512, 3, 5
KK = K * K
N_CORES = 8
RPP = H // 128          # output rows per partition (4)
HALO_ROWS = RPP + K - 1  # input rows held per partition (8)
JW = 520                 # padded row width per copy (covers cols -2..517)
N_WT = 4                 # W tiles
WT_W = W // N_WT         # 128 cols per W tile
PF = C * RPP * WT_W      # product free size per partition (1536)
BANK = 512               # psum bank free size (f32)
NB = PF // BANK          # psum banks per W tile (3)
X_FREE = 2 * C * HALO_ROWS * JW   # 24960 fp16 elems per partition
WT_FREE = KK * RPP * WT_W         # 12800 fp16 elems per partition per W tile


def _split_multi_waits(nc):
    """The walrus build in this container encodes at most ONE sync-wait per
    instruction; Tile's sem-assignment attaches one wait per dependency.
    Hoist the extra waits onto single-wait NOPs inserted just before the
    instruction on the same engine (equivalent sequencer semantics)."""
    n_split = 0
    for f in nc.m.functions:
        for bb in f.blocks:
            new_insts = []
            changed = False
            for inst in bb.instructions:
                si = inst.sync_info
                waits = list(si.on_wait) if (si is not None and si.on_wait) else []
                if len(waits) > 1:
                    changed = True
                    for w in waits[:-1]:
                        nop = mybir.InstNoOp(
                            name=nc.get_next_instruction_name(),
                            engine=inst.engine,
                            sync_info=mybir.SyncInfo(on_wait=[w], on_update=[]),
                            bass_nofuse=True,
                        )
                        new_insts.append(nop)
                        n_split += 1
                    inst.sync_info = mybir.SyncInfo(
                        on_wait=[waits[-1]],
                        on_update=list(si.on_update) if si.on_update else [],
                    )
                new_insts.append(inst)
            if changed:
                bb.instructions = new_insts
    return n_split


_NC_CACHE = None


def _build_program():
    global _NC_CACHE
    if _NC_CACHE is not None:
        return _NC_CACHE

    fp16 = mybir.dt.float16
    f32 = mybir.dt.float32

    nc = bass.Bass("TRN2", target_bir_lowering=False, debug=False,
                   num_devices=N_CORES)
    xbuf = nc.dram_tensor("xbuf", [128, X_FREE], fp16, kind="ExternalInput").ap()
    wtbuf = nc.dram_tensor("wtbuf", [N_WT, 128, WT_FREE], fp16,
                           kind="ExternalInput").ap()
    ident = nc.dram_tensor("ident", [128, 128], fp16, kind="ExternalInput").ap()
    out = nc.dram_tensor("out", [N_WT, 128, PF], fp16, kind="ExternalOutput").ap()

    with TileContext(nc) as tc:
        with (
            tc.tile_pool(name="xpool", bufs=1) as xpool,
            tc.tile_pool(name="wtpool", bufs=2) as wtpool,
            tc.tile_pool(name="prodpool", bufs=6) as prodpool,
            tc.tile_pool(name="outpool", bufs=2) as outpool,
            tc.tile_pool(name="psumpool", bufs=2 * NB, space="PSUM") as psumpool,
        ):
            id_t = xpool.tile([128, 128], fp16)
            nc.sync.dma_start(out=id_t[:], in_=ident[:])
            x_t = xpool.tile([128, X_FREE], fp16)
            nc.sync.dma_start(out=x_t[:], in_=xbuf[:])
            # view: [p, parity, c, row, j]
            xv = x_t[:].rearrange("p (s c r j) -> p s c r j", s=2, c=C, r=HALO_ROWS)

            for t in range(N_WT):
                wt_t = wtpool.tile([128, WT_FREE], fp16)
                nc.sync.dma_start(out=wt_t[:], in_=wtbuf[t])

                ps = [
                    psumpool.tile([128, BANK], f32, tag="ps", name=f"ps_{t}_{b}")
                    for b in range(NB)
                ]

                for k in range(KK):
                    dh, dw = k // K, k % K
                    sel = dw % 2          # parity copy: even dw -> 0, odd -> 1
                    j0 = t * WT_W + dw + sel
                    prod = prodpool.tile([128, PF], fp16)
                    pv = prod[:].rearrange("p (c r w) -> p c r w", c=C, r=RPP)
                    xs = xv[:, sel, :, dh:dh + RPP, j0:j0 + WT_W]
                    wk = (
                        wt_t[:, k * RPP * WT_W:(k + 1) * RPP * WT_W]
                        .rearrange("p (r w) -> p r w", r=RPP)[:, None, :, :]
                        .broadcast_to([128, C, RPP, WT_W])
                    )
                    nc.vector.tensor_mul(out=pv, in0=xs, in1=wk)
                    for b in range(NB):
                        mm = nc.tensor.matmul(
                            ps[b][:],
                            id_t[:],
                            prod[:, b * BANK:(b + 1) * BANK],
                            start=(k == 0),
                            stop=(k == KK - 1),
                        )
                        # The stationary (identity) never changes; skip the
                        # per-matmul reload after the very first load.
                        if not (t == 0 and k == 0 and b == 0):
                            mm.ins.ldweights = False

                o_t = outpool.tile([128, PF], fp16)
                for b in range(NB):
                    nc.scalar.copy(
                        out=o_t[:, b * BANK:(b + 1) * BANK], in_=ps[b][:]
                    )
                nc.sync.dma_start(out=out[t], in_=o_t[:])

    _split_multi_waits(nc)
    _NC_CACHE = nc
    return nc


def _pack_inputs(input_data: np.ndarray, weights: np.ndarray):
    """Host-side layout + fp16 conversion into per-core SBUF-ready buffers."""
    xh = input_data.astype(np.float16)     # (N, H, W, C)
    wh = weights.astype(np.float16)        # (N, H, W, KK)

    in_maps = []
    identity = np.eye(128, dtype=np.float16)
    for n in range(N_CORES):
        # canvas[c, rr, j]: rr = input row + 2 (0..515), col -3.. via j offset
        canvas = np.zeros((C, H + 4, JW + 1), dtype=np.float16)
        canvas[:, 2:2 + H, 3:3 + W] = xh[n].transpose(2, 0, 1)
        rows_a = canvas[:, :, 1:1 + JW]    # A[j] = col j-2
        rows_b = canvas[:, :, 0:JW]        # B[j] = col j-3
        # partition p, halo row rr -> canvas row 4p+rr
        swa = np.lib.stride_tricks.sliding_window_view(rows_a, HALO_ROWS, axis=1)
        swb = np.lib.stride_tricks.sliding_window_view(rows_b, HALO_ROWS, axis=1)
        # (C, 509, JW, 8) -> take every 4th start -> (C, 128, JW, 8)
        swa = swa[:, ::RPP][:, :128]
        swb = swb[:, ::RPP][:, :128]
        # -> (128, parity, C, rr, j)
        X = np.empty((128, 2, C, HALO_ROWS, JW), dtype=np.float16)
        X[:, 0] = swa.transpose(1, 0, 3, 2)
        X[:, 1] = swb.transpose(1, 0, 3, 2)
        X = np.ascontiguousarray(X.reshape(128, X_FREE))

        # WT[t, p, k, r, w] = wh[n, 4p+r, 128t+w, k]
        wv = wh[n].reshape(128, RPP, N_WT, WT_W, KK)
        WT = np.ascontiguousarray(
            wv.transpose(2, 0, 4, 1, 3).reshape(N_WT, 128, WT_FREE)
        )
        in_maps.append({"xbuf": X, "wtbuf": WT, "ident": identity})
    return in_maps


def _unpack_outputs(results) -> np.ndarray:
    out = np.empty((N, H, W, C), dtype=np.float32)
    for n in range(N_CORES):
        o = results[n]["out"].astype(np.float32)  # (N_WT, 128, PF)
        o = o.reshape(N_WT, 128, C, RPP, WT_W)
        # -> h = 4p+r, w = 128t+w, c
        out[n] = o.transpose(1, 3, 0, 4, 2).reshape(H, W, C)
    return out


def kernel(input_data: np.ndarray, weights: np.ndarray) -> np.ndarray:
    input_data = np.asarray(input_data, dtype=np.float32)
    weights = np.asarray(weights, dtype=np.float32)
    nc = _build_program()
    in_maps = _pack_inputs(input_data, weights)
    res = run_bass_kernel_spmd(nc, in_maps, list(range(N_CORES)))
    return _unpack_outputs(res.results)


if __name__ == "__main__":
    rng = np.random.default_rng(0)
    x = rng.standard_normal((N, H, W, C), dtype=np.float32)
    w = rng.standard_normal((N, H, W, KK), dtype=np.float32) * 0.1
    out = kernel(input_data=x, weights=w)
    print("out", out.shape, out.dtype, float(np.abs(out).mean()))
